# revision 34
# baseline (speedup 1.0000x reference)
"""Trainium2 Bass kernel for the MERU-style hyperbolic contrastive loss.

Problem (hardcoded shapes):
  text_embeddings (8192, 768) f32, label_embeddings (4096, 768) f32,
  target_labels (8192,) int32, three scalar log-params.
  Output: loss (8192,) f32 per-sample.

Sharding: data-parallel over text rows across 8 NeuronCores (1024 rows each);
label_embeddings and scalars replicated.

Per-core algorithm (v3 — fp8 DoubleRow matmuls + positive-hoist mask):
  Lorentz inner product factors as  inner[m,c] = hx_m * W[m,c] with
      W[m,c] = g_c*S_raw[m,c] - xtq_m*yt_c,
  where S_raw = raw_text @ raw_labels^T, hx_m/g_c the exp-map scale factors,
  yt_c the label time component and xtq_m = x_time_m / hx_m.  Since hx_m > 0
  is row-constant, per-row top-k runs directly on (SY*W) in PSUM.

   1. Text: quarter-granular f32 load; Pool f32->bf16; K-major fp8 tiles
      via PE identity matmuls (bf16 in, f32 PSUM out) + ACT fp8 copies;
      row norms on DVE from the bf16 copy; Sqrt-free stats (Exp/Ln only,
      single ACT table); xtq split into 3 fp8 levels -> fp8 rank-6 rows.
   2. Labels in 4 pipelined groups of 1024 (quarter-granular loads, 8
      rotating bufs): norms striped ACT/DVE, group stats, Pool f32->bf16,
      then PE matmuls against diag(SY*g_c) fuse the per-label fold with
      the transpose (f32 PSUM -> fp8 SBUF copies on ACT/DVE).
   3. Main loop per (group, m-tile): PSUM accumulates SY*W via 2x(3 fp8
      DoubleRow K=256 matmuls + fp8 DoubleRow rank-6 time term + bf16
      mask matmul).  The mask adds +448*SY*onehot(target), hoisting the
      positive to the top-1 slot: one DVE max8 per (group, m-tile) yields
      both the top negatives (slots 1+) and the positive (slot0 - 448*SY)
      — no label gather / separate positive path needed.  SY=16 scales
      the fp8 label quantization out of the subnormal range.
   4. Merges (max8 over 32 candidates -> [pos+mask, neg1, neg2]) are
      interleaved into the last group's m-loop; subtract the mask, fold
      hx_m/SY in; batched Exp/Ln loss tail.
"""

from contextlib import ExitStack

import numpy as np

import concourse.bass as bass
import concourse.tile as tile
from concourse import bacc, mybir
from concourse import bass_utils

F32 = mybir.dt.float32
BF16 = mybir.dt.bfloat16
FP8 = mybir.dt.float8e4
I32 = mybir.dt.int32
U16 = mybir.dt.uint16
AF = mybir.ActivationFunctionType
ALU = mybir.AluOpType
AX = mybir.AxisListType
DR = mybir.MatmulPerfMode.DoubleRow

N_CORES = 8
M_FULL = 8192
C = 4096
D = 768
M_LOC = M_FULL // N_CORES   # 1024 rows per core
P = 128
NT_M = M_LOC // P           # 8 m-tiles
NT_C = C // P               # 32 label tiles
KCH = D // P                # 6 contraction chunks
NG = 4                      # label groups
GT = NT_C // NG             # 8 label tiles per group
GC = C // NG                # 1024 labels per group
NF = 512
EPS = 1e-8
SY = 16.0                   # fp8 label scale (dodges e4m3 subnormals)
MASK = 448.0 * SY           # positive-hoist mask weight (exact in bf16)


def _stats_chain(nc, pool, nsq_raw, w, alpha_b, curv_b, isqch_b, tg,
                 want_t=False, want_tq=False):
    """From ||raw_row||^2 (128,w) compute gfac = alpha*sinh(rc)/rc plus the
    time component t = cosh(rc)/sqrt(curv) and/or tq = t/gfac, where
    rc = sqrt(curv)*alpha*||raw_row||.  ACT funcs are Exp/Ln only (single
    table); 1/x goes through the exact DVE reciprocal."""
    a2 = pool.tile([P, 1], F32, tag=f"a2{tg}")
    nc.vector.tensor_mul(a2[:], alpha_b[:], alpha_b[:])
    nsq = pool.tile([P, w], F32, tag=f"nsq{tg}")
    nc.vector.tensor_scalar(nsq[:], nsq_raw[:], a2[:], None, op0=ALU.mult)
    rc2 = pool.tile([P, w], F32, tag=f"rc2{tg}")
    nc.vector.tensor_scalar(rc2[:], nsq[:], curv_b[:], None, op0=ALU.mult)
    nc.vector.tensor_scalar_max(rc2[:], rc2[:], EPS * EPS)
    lr2 = pool.tile([P, w], F32, tag=f"lr2{tg}")
    nc.scalar.activation(lr2[:], rc2[:], AF.Ln)
    rc = pool.tile([P, w], F32, tag=f"rc{tg}")
    nc.scalar.activation(rc[:], lr2[:], AF.Exp, scale=0.5)
    rinv = pool.tile([P, w], F32, tag=f"rinv{tg}")
    nc.scalar.activation(rinv[:], lr2[:], AF.Exp, scale=-0.5)
    ep = pool.tile([P, w], F32, tag=f"ep{tg}")
    nc.scalar.activation(ep[:], rc[:], AF.Exp)
    en = pool.tile([P, w], F32, tag=f"en{tg}")
    nc.scalar.activation(en[:], rc[:], AF.Exp, scale=-1.0)
    sh = pool.tile([P, w], F32, tag=f"sh{tg}")
    nc.vector.tensor_sub(sh[:], ep[:], en[:])
    nc.vector.tensor_scalar_mul(sh[:], sh[:], 0.5)
    fac = pool.tile([P, w], F32, tag=f"fac{tg}")
    nc.vector.tensor_mul(fac[:], sh[:], rinv[:])
    gfac = pool.tile([P, w], F32, tag=f"gfac{tg}")
    nc.vector.tensor_scalar(gfac[:], fac[:], alpha_b[:], None, op0=ALU.mult)
    t = None
    if want_t or want_tq:
        # x_time = cosh(dist-from-origin)/sqrt(curv) on the hyperboloid
        t = pool.tile([P, w], F32, tag=f"t{tg}")
        nc.vector.tensor_add(t[:], ep[:], en[:])
        nc.vector.tensor_scalar(t[:], t[:], isqch_b[:], None, op0=ALU.mult)
    tq = None
    if want_tq:
        ginv = pool.tile([P, w], F32, tag=f"ginv{tg}")
        nc.vector.reciprocal(ginv[:], gfac[:])
        tq = pool.tile([P, w], F32, tag=f"tq{tg}")
        nc.vector.tensor_mul(tq[:], t[:], ginv[:])
    return gfac, t, tq


def _split3_fp8(nc, pool, x, w, tg):
    """Split f32 (128,w) into three fp8e4m3 levels h+m+l (residual coding)."""
    levels = []
    res = x
    for lv in range(3):
        q = pool.tile([P, w], FP8, tag=f"q{lv}{tg}")
        nc.vector.tensor_copy(q[:], res[:])
        levels.append(q)
        if lv < 2:
            qf = pool.tile([P, w], F32, tag=f"qf{lv}{tg}")
            nc.vector.tensor_copy(qf[:], q[:])
            nres = pool.tile([P, w], F32, tag=f"qr{lv}{tg}")
            nc.vector.tensor_sub(nres[:], res[:], qf[:])
            res = nres
    return levels


def build_kernel(ctx: ExitStack, tc: tile.TileContext, nt_m: int = NT_M):
    nc = tc.nc

    text_d = nc.dram_tensor("text_embeddings", (M_LOC, D), F32, kind="ExternalInput").ap()
    labels_d = nc.dram_tensor("label_embeddings", (C, D), F32, kind="ExternalInput").ap()
    tgt_d = nc.dram_tensor("target_labels", (M_LOC, 1), I32, kind="ExternalInput").ap()
    curv_log_d = nc.dram_tensor("curv_log", (1, 1), F32, kind="ExternalInput").ap()
    ta_log_d = nc.dram_tensor("text_alpha_log", (1, 1), F32, kind="ExternalInput").ap()
    la_log_d = nc.dram_tensor("label_alpha_log", (1, 1), F32, kind="ExternalInput").ap()
    loss_d = nc.dram_tensor("loss", (M_LOC, 1), F32, kind="ExternalOutput").ap()

    const = ctx.enter_context(tc.tile_pool(name="const", bufs=1))
    tiny = ctx.enter_context(tc.tile_pool(name="tiny", bufs=2))
    junk = ctx.enter_context(tc.tile_pool(name="junk", bufs=2))
    lstage = ctx.enter_context(tc.tile_pool(name="lstage", bufs=2))
    ypool = ctx.enter_context(tc.tile_pool(name="ypool", bufs=2))
    psum = ctx.enter_context(tc.tile_pool(name="psum", bufs=4, space="PSUM"))
    dram = ctx.enter_context(tc.tile_pool(name="dram", bufs=1, space="DRAM"))

    # ---- runtime scalars on the ACT ring (SP ring starts with text) ----
    def bload(ap_d, tag):
        b = const.tile([P, 1], F32, tag=tag)
        nc.scalar.dma_start(b[:], bass.AP(ap_d.tensor, 0, [[0, P], [1, 1]]))
        return b

    cl_b = bload(curv_log_d, "cl_b")
    ta_b = bload(ta_log_d, "ta_b")
    la_b = bload(la_log_d, "la_b")
    curv_b = const.tile([P, 1], F32, tag="curv_b")
    nc.scalar.activation(curv_b[:], cl_b[:], AF.Exp)
    at_b = const.tile([P, 1], F32, tag="at_b")
    nc.scalar.activation(at_b[:], ta_b[:], AF.Exp)
    al_b = const.tile([P, 1], F32, tag="al_b")
    nc.scalar.activation(al_b[:], la_b[:], AF.Exp)
    isqc_b = const.tile([P, 1], F32, tag="isqc_b")
    nc.scalar.activation(isqc_b[:], cl_b[:], AF.Exp, scale=-0.5)
    isqch_b = const.tile([P, 1], F32, tag="isqch_b")  # 0.5/sqrt(curv)
    nc.vector.tensor_scalar_mul(isqch_b[:], isqc_b[:], 0.5)
    ncurv_b = const.tile([P, 1], F32, tag="ncurv_b")
    nc.vector.tensor_scalar_mul(ncurv_b[:], curv_b[:], -1.0)

    # ---- constants + targets (small; issued after text/g0 on their rings)
    iota_u = const.tile([P, C], U16, tag="iota_u")
    nc.gpsimd.iota(iota_u[:], [[1, C]], channel_multiplier=0)
    iota_col = const.tile([P, 1], F32, tag="iota_col")
    nc.gpsimd.iota(
        iota_col[:], [[0, 1]], channel_multiplier=1,
        allow_small_or_imprecise_dtypes=True,
    )
    # maskI[p, i] = +MASK * (i == p): stationary weights of the mask matmul
    # (hoists the positive to the global top-1 candidate slot)
    maskI = const.tile([P, P], BF16, tag="maskI")
    nc.vector.tensor_scalar(
        maskI[:], iota_u[:, 0:P], iota_col[:], MASK, op0=ALU.is_equal,
        op1=ALU.mult,
    )
    # posI: exact 0/1 identity, the permutation operand of PE transposes
    posI = const.tile([P, P], BF16, tag="posI")
    nc.vector.tensor_scalar(
        posI[:], iota_u[:, 0:P], iota_col[:], None, op0=ALU.is_equal,
    )

    # ---- text pass first: its transpose chain gates the first matmul ----
    xstage = const.tile([P, NT_M, D], F32, tag="xstage")
    xbf8 = const.tile([P, NT_M, D], BF16, tag="xbf8")

    def load_group(g, hp=False):
        c0 = g * GC
        src = labels_d[c0:c0 + GC, :].rearrange("(a p) d -> p a d", p=P)
        quarters = []
        with tc.high_priority() if hp else ExitStack():
            for q in range(4):
                labq = lstage.tile([P, GT // 4, D], F32, tag="lab", bufs=8)
                qs = slice(q * GT // 4, (q + 1) * GT // 4)
                nc.sync.dma_start(labq[:], src[:, qs, :])
                quarters.append(labq)
        return quarters

    nq = max(1, nt_m // 4)
    sls = [slice(i, min(i + nq, nt_m)) for i in range(0, nt_m, nq)]
    for sl in sls:
        nc.sync.dma_start(
            xstage[:, sl, :],
            text_d[sl.start * P:sl.stop * P, :].rearrange(
                "(a p) d -> p a d", p=P))
    # first label group load queues on SP right behind the text quarters
    lab_tiles = [load_group(0)]
    for sl in sls:
        with tc.high_priority():
            nc.gpsimd.tensor_copy(xbf8[:, sl, :], xstage[:, sl, :])
    # K-major fp8 text tiles via PE identity-matmul transposes (regular
    # matmul mode: bf16 in, f32 PSUM out, fp8 made by the copies)
    xT_all = const.tile([P, KCH, M_LOC], FP8, tag="xT_all")
    nhh = (nt_m + 3) // 4
    with tc.high_priority():
        for k in range(KCH):
            for h in range(nhh):
                na = min(4, nt_m - 4 * h)
                pst = psum.tile([P, NF], F32, tag="pst", bufs=2)
                for i in range(na):
                    a = 4 * h + i
                    nc.tensor.matmul(
                        pst[:, i * P:(i + 1) * P],
                        xbf8[:, a, k * P:(k + 1) * P], posI[:],
                        start=True, stop=True)
                dst = xT_all[:, k, 4 * h * P:(4 * h + na) * P]
                nc.scalar.activation(dst, pst[:, 0:na * P], AF.Copy)

    # text norms on DVE from the bf16 copy (ACT is busy with xT copies)
    nsqx_raw = const.tile([P, NT_M], F32, tag="nsqx_raw")
    jx = junk.tile([P, D], BF16, tag="jDb")
    for a in range(nt_m):
        nc.vector.scalar_tensor_tensor(
            jx[:], xbf8[:, a, :], 1.0, xbf8[:, a, :],
            op0=ALU.mult, op1=ALU.mult,
            accum_out=nsqx_raw[:, a:a + 1],
        )
    hx, _, xtq = _stats_chain(
        nc, const, nsqx_raw, NT_M, at_b, curv_b, isqch_b, "x", want_tq=True)
    # hx_s folds the 1/SY candidate rescale into the row-constant factor
    hx_s = const.tile([P, NT_M], F32, tag="hx_s")
    nc.vector.tensor_scalar_mul(hx_s[:], hx[:], 1.0 / SY)
    # xtq -> 3 fp8 levels; rank-6 rows [i=0: (h,m,l); i=1: (h,m,h)]
    xlv = _split3_fp8(nc, const, xtq, NT_M, "x")
    xcol_ap = [[1, P], [P, NT_M]]
    xscr = [dram.tile([M_LOC, 1], FP8, tag=f"xt6scr{v}", name=f"xt6scr{v}")
            for v in range(3)]
    xtime6 = const.tile([3, 2, M_LOC], FP8, tag="xtime6")
    with tc.high_priority():
        for v in range(3):
            nc.sync.dma_start(bass.AP(xscr[v][:].tensor, 0, xcol_ap), xlv[v][:])
        for r, v in ((0, 0), (1, 1), (2, 2)):
            nc.sync.dma_start(
                xtime6[r:r + 1, 0, :], xscr[v][:].rearrange("a b -> b a"))
        for r, v in ((0, 0), (1, 1), (2, 0)):
            nc.sync.dma_start(
                xtime6[r:r + 1, 1, :], xscr[v][:].rearrange("a b -> b a"))

    eps24 = const.tile([P, 3 * NT_M], F32, tag="eps24")
    nc.gpsimd.memset(eps24[:], 1.0 + EPS)

    tgt_all = const.tile([P, NT_M], I32, tag="tgt_all")
    nc.sync.dma_start(tgt_all[:], bass.AP(tgt_d.tensor, 0, [[1, P], [P, NT_M]]))
    tgt_f = const.tile([P, NT_M], F32, tag="tgt_f")
    nc.vector.tensor_copy(tgt_f[:], tgt_all[:])

    # persistent per-m-tile state across label groups
    cand = const.tile([P, NT_M * NG * 8], F32, tag="cand")
    V_all = const.tile([P, 3 * NT_M], F32, tag="V_all")
    nsqy_raw = const.tile([P, NT_C], F32, tag="nsqy_raw")
    yscr = [dram.tile([C, 1], FP8, tag=f"yt6scr{v}", name=f"yt6scr{v}")
            for v in range(3)]

    # ---- loss tail in two m-halves (first half overlaps last max8s) ----
    cd = const.tile([P, 3 * NT_M], F32, tag="cd")
    sq = const.tile([P, 3 * NT_M], F32, tag="sqv")
    lsq = const.tile([P, 3 * NT_M], F32, tag="lsq")
    root = const.tile([P, 3 * NT_M], F32, tag="root")
    u = const.tile([P, 3 * NT_M], F32, tag="u")
    dist = const.tile([P, 3 * NT_M], F32, tag="dist")
    ev = const.tile([P, 3 * NT_M], F32, tag="ev")
    s8 = const.tile([P, NT_M], F32, tag="s8")
    lg = const.tile([P, NT_M], F32, tag="lg")
    loss_all = const.tile([P, NT_M], F32, tag="loss_all")

    def loss_tail(m0, m1):
        cs = slice(3 * m0, 3 * m1)
        ms = slice(m0, m1)
        nc.vector.scalar_tensor_tensor(
            cd[:, cs], V_all[:, cs], ncurv_b[:], eps24[:, cs],
            op0=ALU.mult, op1=ALU.max,
        )
        nc.vector.tensor_mul(sq[:, cs], cd[:, cs], cd[:, cs])
        nc.vector.tensor_scalar_add(sq[:, cs], sq[:, cs], -1.0)
        nc.scalar.activation(lsq[:, cs], sq[:, cs], AF.Ln)
        nc.scalar.activation(root[:, cs], lsq[:, cs], AF.Exp, scale=0.5)
        nc.vector.tensor_add(u[:, cs], cd[:, cs], root[:, cs])
        nc.scalar.activation(dist[:, cs], u[:, cs], AF.Ln)
        nc.vector.tensor_scalar(
            dist[:, cs], dist[:, cs], isqc_b[:], None, op0=ALU.mult)
        nc.scalar.activation(ev[:, cs], dist[:, cs], AF.Exp, scale=-1.0)
        ev3 = ev[:, cs].rearrange("p (m k) -> p m k", k=3)
        nc.vector.tensor_reduce(s8[:, ms], ev3, axis=AX.X, op=ALU.add)
        nc.scalar.activation(lg[:, ms], s8[:, ms], AF.Ln)
        dist_pos = dist[:, cs].rearrange("p (m k) -> p m k", k=3)[:, :, 0:1]
        nc.vector.tensor_add(
            loss_all[:, ms], lg[:, ms],
            dist_pos.rearrange("p m k -> p (m k)"),
        )
        nc.sync.dma_start(
            bass.AP(loss_d.tensor, m0 * P, [[1, P], [P, m1 - m0]]),
            loss_all[:, ms],
        )


    # ---- label groups: load -> norms -> stats -> fused fold+fp8 -> yT ----
    for g in range(NG):
        c0 = g * GC
        gsl = slice(g * GT, (g + 1) * GT)
        labq = lab_tiles[g]
        nq4 = GT // 4
        ybf8 = lstage.tile([P, GT, D], BF16, tag="ybf8")
        for q in range(4):
            nc.gpsimd.tensor_copy(
                ybf8[:, q * nq4:(q + 1) * nq4, :], labq[q][:])
        jy = junk.tile([P, D], F32, tag="jD")
        jyb = junk.tile([P, D], BF16, tag="jDb")
        with tc.high_priority():
            for a in range(GT):
                if a % 8 >= 5:
                    nc.vector.scalar_tensor_tensor(
                        jyb[:], ybf8[:, a, :], 1.0, ybf8[:, a, :],
                        op0=ALU.mult, op1=ALU.mult,
                        accum_out=nsqy_raw[:, g * GT + a:g * GT + a + 1],
                    )
                else:
                    nc.scalar.activation(
                        jy[:], labq[a // nq4][:, a % nq4, :], AF.Square,
                        accum_out=nsqy_raw[:, g * GT + a:g * GT + a + 1],
                    )
        with tc.high_priority():
            g_col, yt_col, _ = _stats_chain(
                nc, tiny, nsqy_raw[:, gsl], GT, al_b, curv_b, isqch_b,
                f"y{g}", want_t=True)
            gs_col = tiny.tile([P, GT], F32, tag=f"gs{g}")
            nc.vector.tensor_scalar_mul(gs_col[:], g_col[:], SY)
            ytn = tiny.tile([P, GT], F32, tag=f"ytn{g}")
            nc.vector.tensor_scalar_mul(ytn[:], yt_col[:], -SY)
            ylv = _split3_fp8(nc, tiny, ytn, GT, f"y{g}")

        diagG = tiny.tile([P, GT, P], BF16, tag="diagG")
        for a in range(GT):
            nc.vector.tensor_scalar(
                diagG[:, a, :], posI[:], gs_col[:, a:a + 1], None,
                op0=ALU.mult,
            )
        yTg = ypool.tile([P, KCH, GC], FP8, tag="yTg")
        ytrow6 = ypool.tile([3, 2, GC], FP8, tag="ytrow6")
        with tc.high_priority():
            # time rows first: the DRAM round-trip drains while the folds
            # and transposes run
            col_ap = [[1, P], [P, GT]]
            for v in range(3):
                nc.sync.dma_start(
                    bass.AP(yscr[v][:].tensor, c0, col_ap), ylv[v][:])
            for r, v in ((0, 0), (1, 0), (2, 0)):
                nc.sync.dma_start(
                    ytrow6[r:r + 1, 0, :],
                    yscr[v][c0:c0 + GC, :].rearrange("a b -> b a"))
            for r, v in ((0, 1), (1, 1), (2, 2)):
                nc.sync.dma_start(
                    ytrow6[r:r + 1, 1, :],
                    yscr[v][c0:c0 + GC, :].rearrange("a b -> b a"))

            for k in range(KCH):
                for h in range(2):
                    pst = psum.tile([P, NF], F32, tag="pst", bufs=2)
                    for i in range(4):
                        a = 4 * h + i
                        nc.tensor.matmul(
                            pst[:, i * P:(i + 1) * P],
                            ybf8[:, a, k * P:(k + 1) * P], diagG[:, a, :],
                            start=True, stop=True)
                    dst = yTg[:, k, h * NF:(h + 1) * NF]
                    if g <= 1 and (2 * k + h) % 3 == 2:
                        nc.vector.tensor_copy(dst, pst[:])
                    else:
                        nc.scalar.activation(dst, pst[:], AF.Copy)

        # onehot row blocks for the positive-hoist mask, all built before
        # the m-loop so PE never waits on DVE mid-loop (4x: all 2-byte)
        oh_all = tiny.tile([P, NT_M, GC], BF16, tag="oh_all")
        for m in range(nt_m):
            nc.vector.tensor_scalar(
                oh_all[:, m, :], iota_u[:, c0:c0 + GC], tgt_f[:, m:m + 1],
                None, op0=ALU.is_equal,
            )

        # prefetch up to two groups ahead so each group's post-load chain
        # has a full PE quantum to complete before its matmuls are due
        while len(lab_tiles) < min(g + 2, NG):
            lab_tiles.append(load_group(len(lab_tiles)))

        # ---- main loop for this group ----
        for m in range(nt_m):
            rows = slice(m * P, (m + 1) * P)
            ps = psum.tile([P, GC], F32, tag="ps", bufs=3)
            for h in range(2):
                hs = slice(h * NF, (h + 1) * NF)
                for j in range(KCH // 2):
                    nc.tensor.matmul(
                        ps[:, hs], xT_all[:, 2 * j:2 * j + 2, rows],
                        yTg[:, 2 * j:2 * j + 2, hs],
                        start=(j == 0), stop=False, perf_mode=DR,
                    )
                nc.tensor.matmul(
                    ps[:, hs], xtime6[:, :, rows], ytrow6[:, :, hs],
                    start=False, stop=False, perf_mode=DR,
                )
                nc.tensor.matmul(
                    ps[:, hs], maskI[:], oh_all[:, m, hs],
                    start=False, stop=True,
                )
            # top-8 candidates of this group's SY*W block (PSUM read)
            ci = (m * NG + g) * 8
            nc.vector.max(cand[:, ci:ci + 8], ps[:])
            if g == NG - 1:
                # merge groups -> [pos+MASK, neg1, neg2], fold hx_m/SY in
                top8 = tiny.tile([P, 8], F32, tag="top8")
                nc.vector.max(top8[:], cand[:, m * NG * 8:(m + 1) * NG * 8])
                v3 = tiny.tile([P, 3], F32, tag="v3")
                nc.vector.tensor_scalar_add(v3[:, 0:1], top8[:, 0:1], -MASK)
                nc.vector.tensor_copy(v3[:, 1:3], top8[:, 1:3])
                nc.vector.tensor_scalar(
                    V_all[:, 3 * m:3 * m + 3], v3[:], hx_s[:, m:m + 1], None,
                    op0=ALU.mult,
                )

    loss_tail(0, nt_m)


_CACHED = {}


def _compile_single_act_table(nc):
    """Compile with the act-table insertion pass steered to the one table
    that serves every ACT func this kernel uses (exp/ln/square/copy all live
    in `natural_log_exp_and_others`).  The pass picks the first table
    containing each func, which otherwise thrashes between the exp-only and
    ln-only tables (1.3us per reload).  Table ids stay global, so the NEFF
    loads the real combined table — hardware-correct."""
    import concourse.bacc as bacc_mod
    orig = bacc_mod.get_activation_tables
    keep = "natural_log_exp_and_others"
    ours = {AF.Exp, AF.Ln, AF.Square, AF.Copy}

    def patched(arch):
        tabs = orig(arch)
        return {
            name: (set(s) if name == keep else set(s) - ours)
            for name, s in tabs.items()
        }

    bacc_mod.get_activation_tables = patched
    try:
        nc.compile()
    finally:
        bacc_mod.get_activation_tables = orig


def build_program(nt_m: int = NT_M):
    if nt_m not in _CACHED:
        nc = bacc.Bacc(
            "TRN2",
            target_bir_lowering=False,
            debug=False,
            enable_asserts=False,
            num_devices=N_CORES,
        )
        with tile.TileContext(nc) as tc, ExitStack() as ctx:
            build_kernel(ctx, tc, nt_m)
        _compile_single_act_table(nc)
        _CACHED[nt_m] = nc
    return _CACHED[nt_m]


def shard_inputs(inputs) -> list[dict[str, np.ndarray]]:
    text = np.ascontiguousarray(np.asarray(inputs["text_embeddings"], np.float32))
    labels = np.ascontiguousarray(np.asarray(inputs["label_embeddings"], np.float32))
    tgt = np.asarray(inputs["target_labels"]).astype(np.int32).reshape(M_FULL, 1)
    s11 = lambda v: np.asarray(v, np.float32).reshape(1, 1)
    curv_log = s11(inputs["curv_log"])
    ta = s11(inputs["text_alpha_log"])
    la = s11(inputs["label_alpha_log"])
    in_maps = []
    for i in range(N_CORES):
        r = slice(i * M_LOC, (i + 1) * M_LOC)
        in_maps.append({
            "text_embeddings": np.ascontiguousarray(text[r]),
            "label_embeddings": labels,
            "target_labels": np.ascontiguousarray(tgt[r]),
            "curv_log": curv_log,
            "text_alpha_log": ta,
            "label_alpha_log": la,
        })
    return in_maps


def run_sharded(inputs, trace=False, nt_m: int = NT_M, **kwargs):
    nc = build_program(nt_m)
    in_maps = shard_inputs(inputs)
    res = bass_utils.run_bass_kernel_spmd(
        nc, in_maps, core_ids=list(range(N_CORES)), trace=trace, **kwargs
    )
    loss = np.concatenate(
        [res.results[i]["loss"].reshape(M_LOC) for i in range(N_CORES)]
    ).astype(np.float32)
    return loss, res


def kernel(**inputs) -> np.ndarray:
    loss, _ = run_sharded(inputs, trace=False)
    return loss


# revision 59
# speedup vs baseline: 1.0683x; 1.0683x over previous
"""Trainium2 Bass kernel for the MERU-style hyperbolic contrastive loss.

Problem (hardcoded shapes):
  text_embeddings (8192, 768) f32, label_embeddings (4096, 768) f32,
  target_labels (8192,) int32, three scalar log-params.
  Output: loss (8192,) f32 per-sample.

Sharding: data-parallel over text rows across 8 NeuronCores (1024 rows each);
label_embeddings and scalars replicated.

Per-core algorithm (v3 — fp8 DoubleRow matmuls + positive-hoist mask):
  Lorentz inner product factors as  inner[m,c] = hx_m * W[m,c] with
      W[m,c] = g_c*S_raw[m,c] - xtq_m*yt_c,
  where S_raw = raw_text @ raw_labels^T, hx_m/g_c the exp-map scale factors,
  yt_c the label time component and xtq_m = x_time_m / hx_m.  Since hx_m > 0
  is row-constant, per-row top-k runs directly on (SY*W) in PSUM.

   1. Text: quarter-granular f32 load; Pool f32->bf16; K-major fp8 tiles
      via PE identity matmuls (bf16 in, f32 PSUM out) + ACT fp8 copies;
      row norms on DVE from the bf16 copy; Sqrt-free stats (Exp/Ln only,
      single ACT table); xtq split into 3 fp8 levels -> fp8 rank-6 rows.
   2. Labels in 4 pipelined groups of 1024 (quarter-granular loads, 8
      rotating bufs): norms striped ACT/DVE, group stats, Pool f32->bf16,
      then PE matmuls against diag(SY*g_c) fuse the per-label fold with
      the transpose (f32 PSUM -> fp8 SBUF copies on ACT/DVE).
   3. Main loop per (group, m-tile): PSUM accumulates SY*W via 2x(3 fp8
      DoubleRow K=256 matmuls + fp8 DoubleRow rank-6 time term + bf16
      mask matmul).  The mask adds +448*SY*onehot(target), hoisting the
      positive to the top-1 slot: one DVE max8 per (group, m-tile) yields
      both the top negatives (slots 1+) and the positive (slot0 - 448*SY)
      — no label gather / separate positive path needed.  SY=16 scales
      the fp8 label quantization out of the subnormal range.
   4. Merges (max8 over 32 candidates -> [pos+mask, neg1, neg2]) are
      interleaved into the last group's m-loop; subtract the mask, fold
      hx_m/SY in; batched Exp/Ln loss tail.
"""

from contextlib import ExitStack

import numpy as np

import concourse.bass as bass
import concourse.tile as tile
from concourse import bacc, mybir
from concourse import bass_utils

F32 = mybir.dt.float32
BF16 = mybir.dt.bfloat16
FP8 = mybir.dt.float8e4
I32 = mybir.dt.int32
U16 = mybir.dt.uint16
AF = mybir.ActivationFunctionType
ALU = mybir.AluOpType
AX = mybir.AxisListType
DR = mybir.MatmulPerfMode.DoubleRow

N_CORES = 8
M_FULL = 8192
C = 4096
D = 768
M_LOC = M_FULL // N_CORES   # 1024 rows per core
P = 128
NT_M = M_LOC // P           # 8 m-tiles
NT_C = C // P               # 32 label tiles
KCH = D // P                # 6 contraction chunks
NG = 4                      # label groups
GTS = [8, 8, 8, 8]         # label tiles per group (sum = NT_C)
GTMAX = max(GTS)
T0S = [sum(GTS[:i]) for i in range(NG)]
NF = 512
EPS = 1e-8
SY = 16.0                   # fp8 label scale (dodges e4m3 subnormals)
MASK = 448.0 * SY           # positive-hoist mask weight (exact in bf16)


def _stats_chain(nc, pool, nsq_raw, w, alpha_b, curv_b, isqch_b, tg,
                 want_t=False, want_tq=False):
    """From ||raw_row||^2 (128,w) compute gfac = alpha*sinh(rc)/rc plus the
    time component t = cosh(rc)/sqrt(curv) and/or tq = t/gfac, where
    rc = sqrt(curv)*alpha*||raw_row||.  ACT funcs are Exp/Ln only (single
    table); 1/x goes through the exact DVE reciprocal."""
    a2 = pool.tile([P, 1], F32, tag=f"a2{tg}")
    nc.vector.tensor_mul(a2[:], alpha_b[:], alpha_b[:])
    nsq = pool.tile([P, w], F32, tag=f"nsq{tg}")
    nc.vector.tensor_scalar(nsq[:], nsq_raw[:], a2[:], None, op0=ALU.mult)
    rc2 = pool.tile([P, w], F32, tag=f"rc2{tg}")
    nc.vector.tensor_scalar(rc2[:], nsq[:], curv_b[:], None, op0=ALU.mult)
    nc.vector.tensor_scalar_max(rc2[:], rc2[:], EPS * EPS)
    lr2 = pool.tile([P, w], F32, tag=f"lr2{tg}")
    nc.scalar.activation(lr2[:], rc2[:], AF.Ln)
    rc = pool.tile([P, w], F32, tag=f"rc{tg}")
    nc.scalar.activation(rc[:], lr2[:], AF.Exp, scale=0.5)
    rinv = pool.tile([P, w], F32, tag=f"rinv{tg}")
    nc.scalar.activation(rinv[:], lr2[:], AF.Exp, scale=-0.5)
    ep = pool.tile([P, w], F32, tag=f"ep{tg}")
    nc.scalar.activation(ep[:], rc[:], AF.Exp)
    en = pool.tile([P, w], F32, tag=f"en{tg}")
    nc.scalar.activation(en[:], rc[:], AF.Exp, scale=-1.0)
    sh = pool.tile([P, w], F32, tag=f"sh{tg}")
    nc.vector.tensor_sub(sh[:], ep[:], en[:])
    nc.vector.tensor_scalar_mul(sh[:], sh[:], 0.5)
    fac = pool.tile([P, w], F32, tag=f"fac{tg}")
    nc.vector.tensor_mul(fac[:], sh[:], rinv[:])
    gfac = pool.tile([P, w], F32, tag=f"gfac{tg}")
    nc.vector.tensor_scalar(gfac[:], fac[:], alpha_b[:], None, op0=ALU.mult)
    t = None
    if want_t or want_tq:
        # x_time = cosh(dist-from-origin)/sqrt(curv) on the hyperboloid
        t = pool.tile([P, w], F32, tag=f"t{tg}")
        nc.vector.tensor_add(t[:], ep[:], en[:])
        nc.vector.tensor_scalar(t[:], t[:], isqch_b[:], None, op0=ALU.mult)
    tq = None
    if want_tq:
        ginv = pool.tile([P, w], F32, tag=f"ginv{tg}")
        nc.vector.reciprocal(ginv[:], gfac[:])
        tq = pool.tile([P, w], F32, tag=f"tq{tg}")
        nc.vector.tensor_mul(tq[:], t[:], ginv[:])
    return gfac, t, tq


def _split3_fp8(nc, pool, x, w, tg):
    """Split f32 (128,w) into three fp8e4m3 levels h+m+l (residual coding)."""
    levels = []
    res = x
    for lv in range(3):
        q = pool.tile([P, w], FP8, tag=f"q{lv}{tg}")
        nc.vector.tensor_copy(q[:], res[:])
        levels.append(q)
        if lv < 2:
            qf = pool.tile([P, w], F32, tag=f"qf{lv}{tg}")
            nc.vector.tensor_copy(qf[:], q[:])
            nres = pool.tile([P, w], F32, tag=f"qr{lv}{tg}")
            nc.vector.tensor_sub(nres[:], res[:], qf[:])
            res = nres
    return levels


def build_kernel(ctx: ExitStack, tc: tile.TileContext, nt_m: int = NT_M):
    nc = tc.nc

    text_d = nc.dram_tensor("text_embeddings", (M_LOC, D), F32, kind="ExternalInput").ap()
    labels_d = nc.dram_tensor("label_embeddings", (C, D), F32, kind="ExternalInput").ap()
    tgt_d = nc.dram_tensor("target_labels", (M_LOC, 1), I32, kind="ExternalInput").ap()
    curv_log_d = nc.dram_tensor("curv_log", (1, 1), F32, kind="ExternalInput").ap()
    ta_log_d = nc.dram_tensor("text_alpha_log", (1, 1), F32, kind="ExternalInput").ap()
    la_log_d = nc.dram_tensor("label_alpha_log", (1, 1), F32, kind="ExternalInput").ap()
    loss_d = nc.dram_tensor("loss", (M_LOC, 1), F32, kind="ExternalOutput").ap()

    const = ctx.enter_context(tc.tile_pool(name="const", bufs=1))
    tiny = ctx.enter_context(tc.tile_pool(name="tiny", bufs=2))
    junk = ctx.enter_context(tc.tile_pool(name="junk", bufs=2))
    lstage = ctx.enter_context(tc.tile_pool(name="lstage", bufs=2))
    ypool = ctx.enter_context(tc.tile_pool(name="ypool", bufs=2))
    psum = ctx.enter_context(tc.tile_pool(name="psum", bufs=4, space="PSUM"))
    dram = ctx.enter_context(tc.tile_pool(name="dram", bufs=1, space="DRAM"))

    # ---- runtime scalars on the ACT ring (SP ring starts with text) ----
    def bload(ap_d, tag):
        b = const.tile([P, 1], F32, tag=tag)
        nc.scalar.dma_start(b[:], bass.AP(ap_d.tensor, 0, [[0, P], [1, 1]]))
        return b

    cl_b = bload(curv_log_d, "cl_b")
    ta_b = bload(ta_log_d, "ta_b")
    la_b = bload(la_log_d, "la_b")
    curv_b = const.tile([P, 1], F32, tag="curv_b")
    nc.scalar.activation(curv_b[:], cl_b[:], AF.Exp)
    at_b = const.tile([P, 1], F32, tag="at_b")
    nc.scalar.activation(at_b[:], ta_b[:], AF.Exp)
    al_b = const.tile([P, 1], F32, tag="al_b")
    nc.scalar.activation(al_b[:], la_b[:], AF.Exp)
    isqc_b = const.tile([P, 1], F32, tag="isqc_b")
    nc.scalar.activation(isqc_b[:], cl_b[:], AF.Exp, scale=-0.5)
    isqch_b = const.tile([P, 1], F32, tag="isqch_b")  # 0.5/sqrt(curv)
    nc.vector.tensor_scalar_mul(isqch_b[:], isqc_b[:], 0.5)
    ncurv_b = const.tile([P, 1], F32, tag="ncurv_b")
    nc.vector.tensor_scalar_mul(ncurv_b[:], curv_b[:], -1.0)

    # ---- constants + targets (small; issued after text/g0 on their rings)
    iota_s = const.tile([P, P], U16, tag="iota_s")
    nc.gpsimd.iota(iota_s[:], [[1, P]], channel_multiplier=0)
    iota_col = const.tile([P, 1], F32, tag="iota_col")
    nc.gpsimd.iota(
        iota_col[:], [[0, 1]], channel_multiplier=1,
        allow_small_or_imprecise_dtypes=True,
    )
    # maskI[p, i] = +MASK * (i == p): stationary weights of the mask matmul
    # (hoists the positive to the global top-1 candidate slot)
    maskI = const.tile([P, P], BF16, tag="maskI")
    nc.vector.tensor_scalar(
        maskI[:], iota_s[:], iota_col[:], MASK, op0=ALU.is_equal,
        op1=ALU.mult,
    )
    # posI: exact 0/1 identity, the permutation operand of PE transposes
    posI = const.tile([P, P], BF16, tag="posI")
    nc.vector.tensor_scalar(
        posI[:], iota_s[:], iota_col[:], None, op0=ALU.is_equal,
    )
    # full-width iota for the onehot builds (big: issued after the small one)
    iota_u = const.tile([P, C], U16, tag="iota_u")
    nc.gpsimd.iota(iota_u[:], [[1, C]], channel_multiplier=0)

    # ---- text pass first: its transpose chain gates the first matmul ----
    xstage = const.tile([P, NT_M, D], F32, tag="xstage")
    xbf8 = const.tile([P, NT_M, D], BF16, tag="xbf8")

    def load_group(g, hp=False):
        c0 = T0S[g] * P
        gt = GTS[g]
        src = labels_d[c0:c0 + gt * P, :].rearrange("(a p) d -> p a d", p=P)
        chunks = []
        with tc.high_priority() if hp else ExitStack():
            for q in range(gt // 2):
                labq = lstage.tile([P, 2, D], F32, tag="lab", bufs=8)
                nc.sync.dma_start(labq[:], src[:, 2 * q:2 * q + 2, :])
                chunks.append(labq)
        return chunks

    nq = max(1, nt_m // 4)
    sls = [slice(i, min(i + nq, nt_m)) for i in range(0, nt_m, nq)]
    for sl in sls:
        nc.sync.dma_start(
            xstage[:, sl, :],
            text_d[sl.start * P:sl.stop * P, :].rearrange(
                "(a p) d -> p a d", p=P))
    # first label group load queues on SP right behind the text quarters
    lab_tiles = [load_group(0)]
    for sl in sls:
        with tc.high_priority():
            nc.gpsimd.tensor_copy(xbf8[:, sl, :], xstage[:, sl, :])
    # K-major fp8 text tiles via PE identity-matmul transposes (regular
    # matmul mode: bf16 in, f32 PSUM out, fp8 made by the copies)
    xT_all = const.tile([P, KCH, M_LOC], FP8, tag="xT_all")
    nhh = (nt_m + 3) // 4
    with tc.high_priority():
        for k in range(KCH):
            for h in range(nhh):
                na = min(4, nt_m - 4 * h)
                pst = psum.tile([P, NF], F32, tag="pst", bufs=2)
                for i in range(na):
                    a = 4 * h + i
                    nc.tensor.matmul(
                        pst[:, i * P:(i + 1) * P],
                        xbf8[:, a, k * P:(k + 1) * P], posI[:],
                        start=True, stop=True)
                dst = xT_all[:, k, 4 * h * P:(4 * h + na) * P]
                nc.scalar.activation(dst, pst[:, 0:na * P], AF.Copy)

    # text norms on DVE from the bf16 copy (ACT is busy with xT copies)
    nsqx_raw = const.tile([P, NT_M], F32, tag="nsqx_raw")
    jx = junk.tile([P, D], BF16, tag="jDb")
    for a in range(nt_m):
        nc.vector.scalar_tensor_tensor(
            jx[:], xbf8[:, a, :], 1.0, xbf8[:, a, :],
            op0=ALU.mult, op1=ALU.mult,
            accum_out=nsqx_raw[:, a:a + 1],
        )
    hx, _, xtq = _stats_chain(
        nc, const, nsqx_raw, NT_M, at_b, curv_b, isqch_b, "x", want_tq=True)
    # hx_s folds the 1/SY candidate rescale into the row-constant factor
    hx_s = const.tile([P, NT_M], F32, tag="hx_s")
    nc.vector.tensor_scalar_mul(hx_s[:], hx[:], 1.0 / SY)
    # xtq -> 3 fp8 levels; rank-6 rows [i=0: (h,m,l); i=1: (h,m,h)]
    xlv = _split3_fp8(nc, const, xtq, NT_M, "x")
    xcol_ap = [[1, P], [P, NT_M]]
    xscr = [dram.tile([M_LOC, 1], FP8, tag=f"xt6scr{v}", name=f"xt6scr{v}")
            for v in range(3)]
    xtime6 = const.tile([3, 2, M_LOC], FP8, tag="xtime6")
    for v in range(3):
        nc.sync.dma_start(bass.AP(xscr[v][:].tensor, 0, xcol_ap), xlv[v][:])
    for r, v in ((0, 0), (1, 1), (2, 2)):
        nc.sync.dma_start(
            xtime6[r:r + 1, 0, :], xscr[v][:].rearrange("a b -> b a"))
    for r, v in ((0, 0), (1, 1), (2, 0)):
        nc.sync.dma_start(
            xtime6[r:r + 1, 1, :], xscr[v][:].rearrange("a b -> b a"))

    eps24 = const.tile([P, 3 * NT_M], F32, tag="eps24")
    nc.gpsimd.memset(eps24[:], 1.0 + EPS)

    tgt_all = const.tile([P, NT_M], I32, tag="tgt_all")
    nc.sync.dma_start(tgt_all[:], bass.AP(tgt_d.tensor, 0, [[1, P], [P, NT_M]]))
    tgt_f = const.tile([P, NT_M], F32, tag="tgt_f")
    nc.vector.tensor_copy(tgt_f[:], tgt_all[:])

    # persistent per-m-tile state across label groups
    cand = const.tile([P, NT_M * NG * 8], F32, tag="cand")
    V_all = const.tile([P, 3 * NT_M], F32, tag="V_all")
    nsqy_raw = const.tile([P, NT_C], F32, tag="nsqy_raw")
    yscr = [dram.tile([C, 1], FP8, tag=f"yt6scr{v}", name=f"yt6scr{v}")
            for v in range(3)]

    # ---- loss tail in two m-halves (first half overlaps last max8s) ----
    cd = const.tile([P, 3 * NT_M], F32, tag="cd")
    sq = const.tile([P, 3 * NT_M], F32, tag="sqv")
    lsq = const.tile([P, 3 * NT_M], F32, tag="lsq")
    root = const.tile([P, 3 * NT_M], F32, tag="root")
    u = const.tile([P, 3 * NT_M], F32, tag="u")
    dist = const.tile([P, 3 * NT_M], F32, tag="dist")
    ev = const.tile([P, 3 * NT_M], F32, tag="ev")
    s8 = const.tile([P, NT_M], F32, tag="s8")
    lg = const.tile([P, NT_M], F32, tag="lg")
    loss_all = const.tile([P, NT_M], F32, tag="loss_all")

    def loss_tail(m0, m1):
        cs = slice(3 * m0, 3 * m1)
        ms = slice(m0, m1)
        nc.vector.scalar_tensor_tensor(
            cd[:, cs], V_all[:, cs], ncurv_b[:], eps24[:, cs],
            op0=ALU.mult, op1=ALU.max,
        )
        nc.vector.tensor_mul(sq[:, cs], cd[:, cs], cd[:, cs])
        nc.vector.tensor_scalar_add(sq[:, cs], sq[:, cs], -1.0)
        nc.scalar.activation(lsq[:, cs], sq[:, cs], AF.Ln)
        nc.scalar.activation(root[:, cs], lsq[:, cs], AF.Exp, scale=0.5)
        nc.vector.tensor_add(u[:, cs], cd[:, cs], root[:, cs])
        nc.scalar.activation(dist[:, cs], u[:, cs], AF.Ln)
        nc.vector.tensor_scalar(
            dist[:, cs], dist[:, cs], isqc_b[:], None, op0=ALU.mult)
        nc.scalar.activation(ev[:, cs], dist[:, cs], AF.Exp, scale=-1.0)
        ev3 = ev[:, cs].rearrange("p (m k) -> p m k", k=3)
        nc.vector.tensor_reduce(s8[:, ms], ev3, axis=AX.X, op=ALU.add)
        nc.scalar.activation(lg[:, ms], s8[:, ms], AF.Ln)
        dist_pos = dist[:, cs].rearrange("p (m k) -> p m k", k=3)[:, :, 0:1]
        nc.vector.tensor_add(
            loss_all[:, ms], lg[:, ms],
            dist_pos.rearrange("p m k -> p (m k)"),
        )
        nc.sync.dma_start(
            bass.AP(loss_d.tensor, m0 * P, [[1, P], [P, m1 - m0]]),
            loss_all[:, ms],
        )


    # ---- label groups: load -> norms -> stats -> diag-fold yT ----
    for g in range(NG):
        gt = GTS[g]
        t0 = T0S[g]
        c0 = t0 * P
        gc = gt * P
        nsl = gc // NF          # 512-wide slices of this group
        labq = lab_tiles[g]
        ybf8 = lstage.tile([P, GTMAX, D], BF16, tag="ybf8")
        for q in range(gt // 2):
            nc.gpsimd.tensor_copy(ybf8[:, 2 * q:2 * q + 2, :], labq[q][:])
        jy = junk.tile([P, D], F32, tag="jD")
        jyb = junk.tile([P, D], BF16, tag="jDb")
        with tc.high_priority():
            for a in range(gt):
                if a % 8 >= 5:
                    nc.vector.scalar_tensor_tensor(
                        jyb[:], ybf8[:, a, :], 1.0, ybf8[:, a, :],
                        op0=ALU.mult, op1=ALU.mult,
                        accum_out=nsqy_raw[:, t0 + a:t0 + a + 1],
                    )
                else:
                    nc.scalar.activation(
                        jy[:], labq[a // 2][:, a % 2, :], AF.Square,
                        accum_out=nsqy_raw[:, t0 + a:t0 + a + 1],
                    )
        with tc.high_priority():
            g_col, yt_col, _ = _stats_chain(
                nc, tiny, nsqy_raw[:, t0:t0 + gt], gt, al_b, curv_b, isqch_b,
                f"y{g}", want_t=True)
            gs_col = tiny.tile([P, gt], F32, tag=f"gs{g}")
            nc.vector.tensor_scalar_mul(gs_col[:], g_col[:], SY)
            ytn = tiny.tile([P, gt], F32, tag=f"ytn{g}")
            nc.vector.tensor_scalar_mul(ytn[:], yt_col[:], -SY)
            ylv = _split3_fp8(nc, tiny, ytn, gt, f"y{g}")

        diagG = tiny.tile([P, GTMAX, P], BF16, tag="diagG")
        for a in range(gt):
            nc.scalar.activation(
                diagG[:, a, :], posI[:], AF.Copy, scale=gs_col[:, a:a + 1])
        yTg = ypool.tile([P, KCH, GTMAX * P], FP8, tag="yTg")
        ytrow6 = ypool.tile([3, 2, GTMAX * P], FP8, tag="ytrow6")
        # time rows at NORMAL priority: high priority would park these
        # stats-dependent descriptors at the queue head and stall the
        # label-load stream behind them
        col_ap = [[1, P], [P, gt]]
        for v in range(3):
            nc.sync.dma_start(
                bass.AP(yscr[v][:].tensor, c0, col_ap), ylv[v][:])
        for r, v in ((0, 0), (1, 0), (2, 0)):
            nc.sync.dma_start(
                ytrow6[r:r + 1, 0, 0:gc],
                yscr[v][c0:c0 + gc, :].rearrange("a b -> b a"))
        for r, v in ((0, 1), (1, 1), (2, 2)):
            nc.sync.dma_start(
                ytrow6[r:r + 1, 1, 0:gc],
                yscr[v][c0:c0 + gc, :].rearrange("a b -> b a"))

        with tc.high_priority():
            for k in range(KCH):
                for h, (o0, w) in enumerate(slws):
                    pst = psum.tile([P, NF], F32, tag="pst", bufs=2)
                    for i in range(w // P):
                        a = o0 // P + i
                        nc.tensor.matmul(
                            pst[:, i * P:(i + 1) * P],
                            ybf8[:, a, k * P:(k + 1) * P], diagG[:, a, :],
                            start=True, stop=True)
                    dst = yTg[:, k, o0:o0 + w]
                    if g <= 1 and (2 * k + h) % 3 == 2:
                        nc.vector.tensor_copy(dst, pst[:, 0:w])
                    else:
                        nc.scalar.activation(dst, pst[:, 0:w], AF.Copy)

        # onehot row blocks for the positive-hoist mask, all built before
        # the m-loop so PE never waits on DVE mid-loop (4x: all 2-byte)
        oh_all = tiny.tile([P, NT_M, GTMAX * P], BF16, tag="oh_all")
        for m in range(nt_m):
            nc.vector.tensor_scalar(
                oh_all[:, m, 0:gc], iota_u[:, c0:c0 + gc], tgt_f[:, m:m + 1],
                None, op0=ALU.is_equal,
            )

        # prefetch up to two groups ahead so each group's post-load chain
        # has a full PE quantum to complete before its matmuls are due
        while len(lab_tiles) < min(g + 2, NG):
            lab_tiles.append(load_group(len(lab_tiles)))

        # ---- main loop for this group ----
        for m in range(nt_m):
            rows = slice(m * P, (m + 1) * P)
            ps = psum.tile([P, GTMAX * P], F32, tag="ps", bufs=3)
            for h in range(nsl):
                hs = slice(h * NF, (h + 1) * NF)
                for j in range(KCH // 2):
                    nc.tensor.matmul(
                        ps[:, hs], xT_all[:, 2 * j:2 * j + 2, rows],
                        yTg[:, 2 * j:2 * j + 2, hs],
                        start=(j == 0), stop=False, perf_mode=DR,
                    )
                nc.tensor.matmul(
                    ps[:, hs], xtime6[:, :, rows], ytrow6[:, :, hs],
                    start=False, stop=False, perf_mode=DR,
                )
                nc.tensor.matmul(
                    ps[:, hs], maskI[:], oh_all[:, m, hs],
                    start=False, stop=True,
                )
            # top-8 candidates of this group's SY*W block (PSUM read)
            ci = (m * NG + g) * 8
            nc.vector.max(cand[:, ci:ci + 8], ps[:, 0:gc])
            if g == NG - 1:
                # merge groups -> [pos+MASK, neg1, neg2], fold hx_m/SY in
                # (post-processing on ACT: bias/scale ride the activations)
                top8 = tiny.tile([P, 8], F32, tag="top8")
                nc.vector.max(top8[:], cand[:, m * NG * 8:(m + 1) * NG * 8])
                v3 = tiny.tile([P, 3], F32, tag="v3")
                nc.scalar.activation(
                    v3[:, 0:1], top8[:, 0:1], AF.Copy, bias=-MASK)
                nc.scalar.activation(v3[:, 1:3], top8[:, 1:3], AF.Copy)
                nc.scalar.activation(
                    V_all[:, 3 * m:3 * m + 3], v3[:], AF.Copy,
                    scale=hx_s[:, m:m + 1])

    loss_tail(0, nt_m)


_CACHED = {}


def _compile_single_act_table(nc):
    """Compile with the act-table insertion pass steered to the one table
    that serves every ACT func this kernel uses (exp/ln/square/copy all live
    in `natural_log_exp_and_others`).  The pass picks the first table
    containing each func, which otherwise thrashes between the exp-only and
    ln-only tables (1.3us per reload).  Table ids stay global, so the NEFF
    loads the real combined table — hardware-correct."""
    import concourse.bacc as bacc_mod
    orig = bacc_mod.get_activation_tables
    keep = "natural_log_exp_and_others"
    ours = {AF.Exp, AF.Ln, AF.Square, AF.Copy}

    def patched(arch):
        tabs = orig(arch)
        return {
            name: (set(s) if name == keep else set(s) - ours)
            for name, s in tabs.items()
        }

    bacc_mod.get_activation_tables = patched
    try:
        nc.compile()
    finally:
        bacc_mod.get_activation_tables = orig


def build_program(nt_m: int = NT_M):
    if nt_m not in _CACHED:
        nc = bacc.Bacc(
            "TRN2",
            target_bir_lowering=False,
            debug=False,
            enable_asserts=False,
            num_devices=N_CORES,
        )
        with tile.TileContext(nc) as tc, ExitStack() as ctx:
            build_kernel(ctx, tc, nt_m)
        _compile_single_act_table(nc)
        _CACHED[nt_m] = nc
    return _CACHED[nt_m]


def shard_inputs(inputs) -> list[dict[str, np.ndarray]]:
    text = np.ascontiguousarray(np.asarray(inputs["text_embeddings"], np.float32))
    labels = np.ascontiguousarray(np.asarray(inputs["label_embeddings"], np.float32))
    tgt = np.asarray(inputs["target_labels"]).astype(np.int32).reshape(M_FULL, 1)
    s11 = lambda v: np.asarray(v, np.float32).reshape(1, 1)
    curv_log = s11(inputs["curv_log"])
    ta = s11(inputs["text_alpha_log"])
    la = s11(inputs["label_alpha_log"])
    in_maps = []
    for i in range(N_CORES):
        r = slice(i * M_LOC, (i + 1) * M_LOC)
        in_maps.append({
            "text_embeddings": np.ascontiguousarray(text[r]),
            "label_embeddings": labels,
            "target_labels": np.ascontiguousarray(tgt[r]),
            "curv_log": curv_log,
            "text_alpha_log": ta,
            "label_alpha_log": la,
        })
    return in_maps


def run_sharded(inputs, trace=False, nt_m: int = NT_M, **kwargs):
    nc = build_program(nt_m)
    in_maps = shard_inputs(inputs)
    res = bass_utils.run_bass_kernel_spmd(
        nc, in_maps, core_ids=list(range(N_CORES)), trace=trace, **kwargs
    )
    loss = np.concatenate(
        [res.results[i]["loss"].reshape(M_LOC) for i in range(N_CORES)]
    ).astype(np.float32)
    return loss, res


def kernel(**inputs) -> np.ndarray:
    loss, _ = run_sharded(inputs, trace=False)
    return loss
